# revision 32
# baseline (speedup 1.0000x reference)
"""Trainium2 Bass kernel for nn_LossRecovery (spatial+temporal channel attention).

Sharding: 16 (b,l) slices over 8 cores, 2 slices/core (data-parallel over B*L,
small CxC weights replicated per core), per the sharding hint.

v3 design:
- ONE packed input dram tensor + ONE output tensor per core (per-buffer PJRT
  dispatch overhead ~65us/buffer/call dominates at this kernel size).
- Scores via Gram matrices: scores = Wq.(X.X^T).Wk^T + rank-1 bias
  corrections (bq (x) S_k0 + S_q0 (x) bk + HW bq (x) bk, with S = W.xsum).
  This replaces the q/k convs and all their PSUM->SBUF move+bias traffic
  with a single Gram accumulation per attention. Temporal scores2 use the
  cross-Gram of x1 and x-swapped the same way (softmax logits identical).
- fp16 for all activation tensors from HBM and the output; fp32 PSUM,
  fp32r for the small (256x256) Gram-stage matmuls, fp32 softmax.
- x1 lives only in natural (s,c) bf16 layout; output written natural.
- Engine balance: per-partition biases (v, v2) fused into ACT Identity
  copies; v2p drains split ACT/DVE; residual fuses on DVE.

Temporal-value scramble handling (same math as the previous kernel): v2p row
r=32p+co of slice j holds v2conv[l=(l0+p)%8][band-channel 32j+co]; tk weight
ROWS are permuted host-side (dperm) so attn2 columns match v2p rows directly.
"""
import numpy as np

import concourse.bass as bass
import concourse.bacc as bacc
import concourse.mybir as mybir
import concourse.tile as tile
from concourse.bass_utils import run_bass_kernel_spmd
from concourse.masks import make_identity

B, L, H, W = 2, 8, 64, 64
C, HW = 256, 4096
FP = mybir.dt.float32
BF = mybir.dt.float16
FR = mybir.dt.float32r
NS512 = HW // 512   # 8
NS128 = HW // 128   # 32
AW = 264            # augmented natural-layout width (256 + ones col + pad)
BF16 = np.float16

_CACHE = {}

# ---- packed input layout (f32 words) ----
_LAYOUT = [
    ("wqk32", 256 * 512),       # [sq_w.T | sk_w.T] (256e, 512) f32
    ("wt232", 256 * 512),       # [tq_w.T | tk_w[dperm].T] (256e, 512) f32
    ("wv", 256 * 128),          # sv_w.T (256c, 256d) bf16
    ("wv2", 256 * 32),          # tv_w[band].T (256c, 64) bf16
    ("rows", 1536),             # [bq|bk|bk4096|bq2|bk2|bk2_4096] (1,1536) f32
    ("vb", 256),                # sv_b (256,1) f32
    ("v2b", 64),                # tv_b[band] (64,1) f32
    ("gam", 256),               # gammas broadcast (128,2) f32
    ("idg", 2 * 128 * 64),      # [I/gs ; I/gt] (2,128,128) fp16
    ("xb", 8 * 256 * 2048),     # (8,256,4096) bf16 (c,s), rotated l order
    ("xnat", 2 * 4096 * (AW // 2)),   # (2,4096,264) bf16 nat rows + ones col
    ("xswnat", 2 * 4096 * (AW // 2)), # (2,4096,264) bf16 w-major rows + ones
]
_OFF = {}
_cur = 0
for _n, _sz in _LAYOUT:
    _OFF[_n] = _cur
    _cur += _sz
NWORDS = _cur


def build_program():
    nc = bacc.Bacc("TRN2", target_bir_lowering=False, debug=False, num_devices=8)

    inp = nc.dram_tensor("inp", [NWORDS], FP, kind="ExternalInput")
    out_d = nc.dram_tensor("out", [2, HW, C], BF, kind="ExternalOutput")

    def reg2(name, rows_, wwords):
        o = _OFF[name]
        return inp[o:o + rows_ * wwords].rearrange("(p n) -> p n", p=rows_)

    def reg3(name, rows_, wwords):
        o = _OFF[name]
        return inp[o:o + rows_ * wwords].rearrange(
            "(cc p n) -> p cc n", p=128, n=wwords)

    def xb_chunk(p, cc):
        o = _OFF["xb"] + (p * 256 + cc * 128) * 2048
        return inp[o:o + 128 * 2048].rearrange("(p n) -> p n", p=128)

    def nat_chunks(name, j):
        # (4096, 264w/2) f32 words -> (128, 32, 132) partition-major source AP
        o = _OFF[name] + j * 4096 * (AW // 2)
        return inp[o:o + 4096 * (AW // 2)].rearrange(
            "(s1 p n) -> p s1 n", p=128, n=AW // 2)

    with tile.TileContext(nc) as tc:
        with (
            tc.tile_pool(name="const", bufs=1) as cpool,
            tc.tile_pool(name="big", bufs=1) as big,
            tc.tile_pool(name="swp", bufs=1) as swp,
            tc.tile_pool(name="xbs", bufs=3) as xbs_pool,
            tc.tile_pool(name="gsb", bufs=2) as gsb,
            tc.tile_pool(name="sm", bufs=2) as sm,
            tc.tile_pool(name="rowp", bufs=2) as rowp,
            tc.tile_pool(name="outp", bufs=4) as outp,
            tc.tile_pool(name="psA", bufs=2, space="PSUM") as psA,
            tc.tile_pool(name="psS2", bufs=2, space="PSUM") as psS2,
            tc.tile_pool(name="psO", bufs=2, space="PSUM") as psO,
            tc.tile_pool(name="psG", bufs=2, space="PSUM") as psG,
        ):
            # ---- first-needed loads: xnat0 chunks, v weights ----
            xnat = [big.tile([128, NS128, AW], BF, tag=f"xnat{j}",
                             name=f"xnat{j}") for j in range(2)]
            src0 = nat_chunks("xnat", 0)
            nc.sync.dma_start(xnat[0][:, bass.ts(0, 8), :],
                              src0[:, bass.ts(0, 8), :].bitcast(BF))
            wv = cpool.tile([128, 2, 256], BF, tag="wv")
            nc.sync.dma_start(wv[:], reg3("wv", 256, 128).bitcast(BF))
            wv2 = cpool.tile([128, 2, 64], BF, tag="wv2")
            nc.sync.dma_start(wv2[:], reg3("wv2", 256, 32).bitcast(BF))
            vb = cpool.tile([128, 2, 1], FP, tag="vb")
            nc.sync.dma_start(vb[:], reg3("vb", 256, 1))
            v2b = cpool.tile([64, 1], FP, tag="v2b")
            nc.sync.dma_start(v2b[:], reg2("v2b", 64, 1))
            gam = cpool.tile([128, 2], FP, tag="gam")
            nc.sync.dma_start(gam[:], reg2("gam", 128, 2))
            ident = cpool.tile([128, 128], FP, tag="ident")
            make_identity(nc, ident[:])
            idg = cpool.tile([128, 2, 128], BF, tag="idg")
            nc.sync.dma_start(idg[:], reg3("idg", 256, 64).bitcast(BF))
            for hh in range(1, 4):
                nc.sync.dma_start(xnat[0][:, bass.ts(hh, 8), :],
                                  src0[:, bass.ts(hh, 8), :].bitcast(BF))
            # deferred loads (needed later; emitted after so the DMA queue
            # services the startup-critical tiles first)
            wqk = cpool.tile([128, 2, 512], FR, tag="wqk")
            wt2 = cpool.tile([128, 2, 512], FR, tag="wt2")
            rowsFR = cpool.tile([1, 1536], FR, tag="rowsFR")

            def load_deferred():
                nc.sync.dma_start(wqk[:], reg3("wqk32", 256, 512).bitcast(FR))
                nc.sync.dma_start(rowsFR[:], reg2("rows", 1, 1536).bitcast(FR))
                src1 = nat_chunks("xnat", 1)
                for hh in range(4):
                    nc.sync.dma_start(xnat[1][:, bass.ts(hh, 8), :],
                                      src1[:, bass.ts(hh, 8), :].bitcast(BF))
                nc.sync.dma_start(wt2[:], reg3("wt232", 256, 512).bitcast(FR))

            g_s = gam[:, 0:1]
            g_t = gam[:, 1:2]
            rFR = rowsFR

            x1n = [big.tile([128, NS128, 256], BF, tag=f"x1n{j}",
                            name=f"x1n{j}") for j in range(2)]
            vt = [big.tile([128, 2, HW], BF, tag=f"vt{j}", name=f"vt{j}")
                  for j in range(2)]
            v2p = [big.tile([128, 2, HW], BF, tag=f"v2p{j}", name=f"v2p{j}")
                   for j in range(2)]

            def load_xswnat(j):
                t = swp.tile([128, NS128, AW], BF, tag="xswnat")
                src = nat_chunks("xswnat", j)
                for hh in range(4):
                    nc.sync.dma_start(t[:, bass.ts(hh, 8), :],
                                      src[:, bass.ts(hh, 8), :].bitcast(BF))
                return t

            def emit_gram(lhs_tile, rhs_tile, gpair):
                """gpair[ec] (128,264) psum += lhsT_chunk^T @ rhs over all s.
                rhs col 256 is 1.0 -> col 256 = row sums of lhs. 32 units."""
                for s1 in range(NS128):
                    for ec in range(2):
                        nc.tensor.matmul(
                            gpair[ec][:],
                            lhs_tile[:, s1, bass.ds(128 * ec, 128)],
                            rhs_tile[:, s1, :],
                            start=(s1 == 0), stop=(s1 == NS128 - 1))
                    yield

            def emit_scores_softmax(gpair, wst, roff, attnT, tagn,
                                    temporal=False, xs_sp=None, xs_keep=None):
                """Drain G (+ sum col); S-rows; two (256x256) stages;
                scores += bq(x)R1 + S_q(x)bk; softmax -> attnT.
                wst: (128,2,512) FR = [WqT | WkT] (e-part, echunk, c|d).
                roff: word offset of [bq|bk|bk*HW] inside `rows`.
                Spatial (G symmetric): T1 = G.WkT via lhsT=G, then
                scores = lhsT(WqT-chunk).T1.
                Temporal (G2 = x1^T.Xsw asym): U^T = lhsT(G2).Wq2T, then
                scores2 = lhsT(U^T-chunk).Wk2T; S_k2 row from spatial
                xsum column xs_sp (sum over xsw == sum over x)."""
                Gt = gsb.tile([128, 2, 256], FR, tag="G", name=f"G_{tagn}")
                xs = sm.tile([128, 2, 1], FR, tag="xsum", name=f"xs_{tagn}")
                for ec in range(2):
                    nc.scalar.copy(Gt[:, ec, :], gpair[ec][:, 0:256])
                    nc.scalar.copy(xs[:, ec, :], gpair[ec][:, 256:257])
                    if xs_keep is not None:
                        nc.scalar.copy(xs_keep[:, ec, :],
                                       gpair[ec][:, 256:257])
                yield
                ps_s = psS2.tile([128, 512], FP, tag="mmS",
                                 name=f"srow_{tagn}")
                if not temporal:
                    # (1,512) = xsum^T . [WqT|WkT] = [S_q row | S_k row]
                    nc.tensor.matmul(ps_s[0:1, :], xs[:, 0, :],
                                     wst[:, 0, :], start=True, stop=False)
                    nc.tensor.matmul(ps_s[0:1, :], xs[:, 1, :],
                                     wst[:, 1, :], start=False, stop=True)
                else:
                    # S_q2 = x1sum^T.Wq2T ; S_k2 = xsum^T.Wk2T
                    nc.tensor.matmul(ps_s[0:1, 0:256], xs[:, 0, :],
                                     wst[:, 0, 0:256], start=True, stop=False)
                    nc.tensor.matmul(ps_s[0:1, 0:256], xs[:, 1, :],
                                     wst[:, 1, 0:256], start=False, stop=True)
                    nc.tensor.matmul(ps_s[0:1, 256:512], xs_sp[:, 0, :],
                                     wst[:, 0, 256:512], start=True, stop=False)
                    nc.tensor.matmul(ps_s[0:1, 256:512], xs_sp[:, 1, :],
                                     wst[:, 1, 256:512], start=False, stop=True)
                sq_row = rowp.tile([1, 256], FR, tag="sqrow",
                                   name=f"sq_{tagn}")
                nc.scalar.copy(sq_row[:], ps_s[0:1, 0:256])
                # R1 = S_k row + HW*bk row
                r1 = rowp.tile([1, 256], FR, tag="r1", name=f"r1_{tagn}")
                nc.vector.tensor_add(r1[:], ps_s[0:1, 256:512],
                                     rowsFR[:, bass.ds(roff + 512, 256)])
                yield
                # stage 1: T1 = lhsT(G).W2 where W2 = WkT (spatial) or
                # WqT (temporal, giving U^T)
                w1off = 256 if not temporal else 0
                t1 = gsb.tile([128, 2, 256], FR, tag="T1", name=f"T1_{tagn}")
                for ec in range(2):
                    pt1 = psS2.tile([128, 512], FP, tag="mmS")
                    nc.tensor.matmul(pt1[:, 0:256],
                                     Gt[:, 0, bass.ds(128 * ec, 128)],
                                     wst[:, 0, bass.ds(w1off, 256)],
                                     start=True, stop=False)
                    nc.tensor.matmul(pt1[:, 0:256],
                                     Gt[:, 1, bass.ds(128 * ec, 128)],
                                     wst[:, 1, bass.ds(w1off, 256)],
                                     start=False, stop=True)
                    nc.scalar.copy(t1[:, ec, :], pt1[:, 0:256])
                    yield
                # stage 2 + corrections + softmax per c-chunk
                for cc in range(2):
                    psc = psS2.tile([128, 512], FP, tag="mmS")
                    sc = psc[:, 0:256]
                    if not temporal:
                        nc.tensor.matmul(sc,
                                         wst[:, 0, bass.ds(128 * cc, 128)],
                                         t1[:, 0, :],
                                         start=True, stop=False)
                        nc.tensor.matmul(sc,
                                         wst[:, 1, bass.ds(128 * cc, 128)],
                                         t1[:, 1, :],
                                         start=False, stop=False)
                    else:
                        nc.tensor.matmul(sc,
                                         t1[:, 0, bass.ds(128 * cc, 128)],
                                         wst[:, 0, 256:512],
                                         start=True, stop=False)
                        nc.tensor.matmul(sc,
                                         t1[:, 1, bass.ds(128 * cc, 128)],
                                         wst[:, 1, 256:512],
                                         start=False, stop=False)
                    nc.tensor.matmul(sc,
                                     rFR[:, bass.ds(roff + 128 * cc, 128)],
                                     r1[:],
                                     start=False, stop=False)
                    nc.tensor.matmul(sc,
                                     sq_row[:, bass.ds(128 * cc, 128)],
                                     rFR[:, bass.ds(roff + 256, 256)],
                                     start=False, stop=True)
                    mx = sm.tile([128, 1], FP, tag="mx")
                    nc.vector.reduce_max(mx[:], sc, axis=mybir.AxisListType.X)
                    nmx = sm.tile([128, 1], FP, tag="nmx")
                    nc.vector.tensor_scalar_mul(nmx[:], mx[:], -1.0)
                    aexp = sm.tile([128, 256], FP, tag="aexp")
                    ssum = sm.tile([128, 1], FP, tag="ssum")
                    nc.scalar.activation(out=aexp[:], in_=sc,
                                         func=mybir.ActivationFunctionType.Exp,
                                         bias=nmx[:], accum_out=ssum[:])
                    rs = sm.tile([128, 1], FP, tag="rs")
                    nc.vector.reciprocal(rs[:], ssum[:])
                    nc.vector.tensor_scalar_mul(aexp[:], aexp[:], rs[:])
                    for dc in range(2):
                        pt = psO.tile([128, 256], FP, tag="mmO")
                        nc.tensor.transpose(pt[:, 0:128],
                                            aexp[:, bass.ts(dc, 128)],
                                            ident[:])
                        nc.scalar.copy(attnT[:, dc, bass.ts(cc, 128)],
                                       pt[:, 0:128])
                    yield

            def emit_phase0(ps):
                """64-band temporal-v conv for source slices in `ps` -> v2p;
                for p<2 the v conv for slice p is fused in (same stream).
                xb streamed in half-slices. Drains split ACT(j0)/DVE(j1)."""
                for p in ps:
                    for h in range(2):
                        ht = xbs_pool.tile([128, 2, 2048], BF, tag="xbs")
                        for cc in range(2):
                            nc.sync.dma_start(
                                ht[:, cc, :],
                                xb_chunk(p, cc)[:, bass.ds(1024 * h, 1024)
                                                ].bitcast(BF))
                        for sq in range(4):
                            s5 = 4 * h + sq
                            rhs0 = ht[:, 0, bass.ts(sq, 512)]
                            rhs1 = ht[:, 1, bass.ts(sq, 512)]
                            ph = psA.tile([64, 512], FP, tag="mmA")
                            nc.tensor.matmul(ph[:], wv2[:, 0, :], rhs0,
                                             start=True, stop=False)
                            nc.tensor.matmul(ph[:], wv2[:, 1, :], rhs1,
                                             start=False, stop=True)
                            nc.scalar.activation(
                                out=v2p[0][bass.ds(32 * (p % 4), 32), p // 4,
                                           bass.ts(s5, 512)],
                                in_=ph[0:32, :],
                                func=mybir.ActivationFunctionType.Identity,
                                bias=v2b[0:32, 0:1])
                            nc.vector.tensor_scalar_add(
                                v2p[1][bass.ds(32 * (p % 4), 32), p // 4,
                                       bass.ts(s5, 512)],
                                ph[32:64, :], v2b[32:64, 0:1])
                            if p < 2:
                                for dc in range(2):
                                    pv = psA.tile([128, 512], FP, tag="mmA")
                                    nc.tensor.matmul(
                                        pv[:], wv[:, 0, bass.ts(dc, 128)],
                                        rhs0, start=True, stop=False)
                                    nc.tensor.matmul(
                                        pv[:], wv[:, 1, bass.ts(dc, 128)],
                                        rhs1, start=False, stop=True)
                                    nc.vector.tensor_scalar_add(
                                        vt[p][:, dc, bass.ts(s5, 512)],
                                        pv[:], vb[:, dc, 0:1])
                            yield

            def emit_out1(j, attnT):
                """spatial out (s,c) + residual -> x1n[j] (fp16). 32 units.
                Alternates drain engine per chunk: even -> DVE fused stt;
                odd -> residual I/gs matmul + ACT scale-by-gs copy."""
                for s1 in range(NS128):
                    po = psO.tile([128, 256], FP, tag="mmO")
                    act_side = s1 % 2 == 1
                    if act_side:
                        nc.tensor.matmul(po[:], idg[:, 0, :],
                                         xnat[j][:, s1, 0:256],
                                         start=True, stop=False)
                    nc.tensor.matmul(po[:],
                                     vt[j][:, 0, bass.ts(s1, 128)],
                                     attnT[:, 0, :], start=not act_side,
                                     stop=False)
                    nc.tensor.matmul(po[:],
                                     vt[j][:, 1, bass.ts(s1, 128)],
                                     attnT[:, 1, :], start=False, stop=True)
                    if act_side:
                        nc.scalar.activation(
                            out=x1n[j][:, s1, :], in_=po[:],
                            func=mybir.ActivationFunctionType.Identity,
                            scale=g_s)
                    else:
                        nc.vector.scalar_tensor_tensor(
                            out=x1n[j][:, s1, :], in0=po[:], scalar=g_s,
                            in1=xnat[j][:, s1, 0:256],
                            op0=mybir.AluOpType.mult,
                            op1=mybir.AluOpType.add)
                    yield

            def emit_out2(j, attn2T):
                """temporal out (s,c) + residual -> x2 -> DMA out. 32 units.
                Same alternating drain-engine scheme as emit_out1."""
                for s1 in range(NS128):
                    po = psO.tile([128, 256], FP, tag="mmO")
                    act_side = s1 % 2 == 1
                    if act_side:
                        nc.tensor.matmul(po[:], idg[:, 1, :],
                                         x1n[j][:, s1, :],
                                         start=True, stop=False)
                    nc.tensor.matmul(po[:],
                                     v2p[j][:, 0, bass.ts(s1, 128)],
                                     attn2T[:, 0, :], start=not act_side,
                                     stop=False)
                    nc.tensor.matmul(po[:],
                                     v2p[j][:, 1, bass.ts(s1, 128)],
                                     attn2T[:, 1, :], start=False, stop=True)
                    x2 = outp.tile([128, 256], BF, tag="x2")
                    if act_side:
                        nc.scalar.activation(
                            out=x2[:], in_=po[:],
                            func=mybir.ActivationFunctionType.Identity,
                            scale=g_t)
                    else:
                        nc.vector.scalar_tensor_tensor(
                            out=x2[:], in0=po[:], scalar=g_t,
                            in1=x1n[j][:, s1, :],
                            op0=mybir.AluOpType.mult,
                            op1=mybir.AluOpType.add)
                    nc.sync.dma_start(
                        out_d[j, bass.ds(128 * s1, 128), :], x2[:])
                    yield

            def weave(a, b, ratio=1):
                ita, itb = iter(a), iter(b)
                alive_a = alive_b = True
                while alive_a or alive_b:
                    if alive_a:
                        try:
                            next(ita)
                        except StopIteration:
                            alive_a = False
                    if alive_b:
                        for _ in range(ratio):
                            try:
                                next(itb)
                            except StopIteration:
                                alive_b = False
                                break

            def chain(*gens):
                for g in gens:
                    yield from g

            def drain(g):
                for _ in g:
                    pass

            # ================= emission order =================
            gA0 = [psG.tile([128, AW], FP, tag="gram", name=f"gA0_{e}")
                   for e in range(2)]
            weave(emit_phase0(range(2)), emit_gram(xnat[0], xnat[0], gA0), 2)
            load_deferred()
            attnT0 = sm.tile([128, 2, 256], BF, tag="attnT", name="attnT0")
            xs_s0 = sm.tile([128, 2, 1], FR, tag="xsC", name="xsc_s0")
            ph0 = emit_phase0(range(2, L))
            weave(emit_scores_softmax(gA0, wqk, 0, attnT0, "s0",
                                      xs_keep=xs_s0), ph0, 2)
            gA1 = [psG.tile([128, AW], FP, tag="gram", name=f"gA1_{e}")
                   for e in range(2)]
            weave(emit_gram(xnat[1], xnat[1], gA1),
                  chain(emit_out1(0, attnT0), ph0), 1)
            attnT1 = sm.tile([128, 2, 256], BF, tag="attnT", name="attnT1")
            xs_s1 = sm.tile([128, 2, 1], FR, tag="xsC", name="xsc_s1")
            weave(emit_scores_softmax(gA1, wqk, 0, attnT1, "s1",
                                      xs_keep=xs_s1), ph0, 2)
            xsw0 = load_xswnat(0)
            gB0 = [psG.tile([128, AW], FP, tag="gram", name=f"gB0_{e}")
                   for e in range(2)]
            o11 = emit_out1(1, attnT1)
            weave(emit_gram(x1n[0], xsw0, gB0), o11, 1)
            attn2T0 = sm.tile([128, 2, 256], BF, tag="attnT", name="attn2T0")
            weave(emit_scores_softmax(gB0, wt2, 768, attn2T0, "t0",
                                      temporal=True, xs_sp=xs_s0),
                  chain(o11, ph0), 2)
            xsw1 = load_xswnat(1)
            o20 = emit_out2(0, attn2T0)
            gB1 = [psG.tile([128, AW], FP, tag="gram", name=f"gB1_{e}")
                   for e in range(2)]
            weave(emit_gram(x1n[1], xsw1, gB1), chain(ph0, o20), 1)
            attn2T1 = sm.tile([128, 2, 256], BF, tag="attnT", name="attn2T1")
            weave(emit_scores_softmax(gB1, wt2, 768, attn2T1, "t1",
                                      temporal=True, xs_sp=xs_s1),
                  chain(ph0, o20), 2)
            drain(ph0)
            weave(emit_out2(1, attn2T1), o20, 1)

    nc.compile()
    return nc


def _tobf_words(a):
    """float32 array -> fp16 packed into f32 words (last dim halved)."""
    b = np.ascontiguousarray(np.asarray(a, np.float32)).astype(BF16)
    return b.view(np.float32)


def _aug(nat):
    """(4096,256) -> (4096,264): ones col at 256, zero pad."""
    out = np.zeros((HW, AW), np.float32)
    out[:, :C] = nat
    out[:, C] = 1.0
    return out


def _prep_core_inputs(x_s, w, k):
    """Host-side packing for core k. x_s: (2,8,64,64,256) fp32."""
    b, q = k // 4, k % 4
    l0 = 2 * q
    band = 32 * l0
    rr = np.arange(C)
    dperm = 8 * (rr % 32) + ((l0 + rr // 32) % 8)
    xb_full = x_s[b]  # (8,64,64,256)
    f32 = np.float32
    rows = np.concatenate([
        w["sq_b"], w["sk_b"], float(HW) * w["sk_b"],
        w["tq_b"], w["tk_b"][dperm],
        float(HW) * w["tk_b"][dperm]]).reshape(1, -1)
    pieces = {
        "wqk32": np.ascontiguousarray(
            np.concatenate([w["sq_w"].T, w["sk_w"].T], axis=1), f32),
        "wt232": np.ascontiguousarray(
            np.concatenate([w["tq_w"].T, w["tk_w"][dperm].T], axis=1), f32),
        "wv": _tobf_words(w["sv_w"].T),
        "wv2": _tobf_words(w["tv_w"][band:band + 64].T),
        "rows": np.ascontiguousarray(rows, f32),
        "vb": np.ascontiguousarray(w["sv_b"].reshape(C, 1), f32),
        "v2b": np.ascontiguousarray(
            w["tv_b"][band:band + 64].reshape(64, 1), f32),
        "gam": np.ascontiguousarray(np.broadcast_to(
            np.stack([w["s_gamma"][0], w["t_gamma"][0]]), (128, 2)), f32),
        "idg": _tobf_words(np.stack(
            [np.eye(128, dtype=f32) / w["s_gamma"][0],
             np.eye(128, dtype=f32) / w["t_gamma"][0]])),
        # xb rotated: row p holds slice l=(l0+p)%8 in (c,s) layout
        "xb": _tobf_words(np.stack(
            [xb_full[(l0 + p) % 8].transpose(2, 0, 1).reshape(C, HW)
             for p in range(L)])),
        "xnat": _tobf_words(np.stack(
            [_aug(xb_full[l].reshape(HW, C)) for l in (l0, l0 + 1)])),
        "xswnat": _tobf_words(np.stack(
            [_aug(xb_full[l].transpose(1, 0, 2).reshape(HW, C))
             for l in (l0, l0 + 1)])),
    }
    buf = np.empty(NWORDS, f32)
    for name, sz in _LAYOUT:
        o = _OFF[name]
        buf[o:o + sz] = pieces[name].reshape(-1)
    return {"inp": buf}


def kernel(**inputs):
    x = np.asarray(inputs["x"], np.float32)
    x_s = np.ascontiguousarray(x[..., :C])
    wnames = ["sq_w", "sq_b", "sk_w", "sk_b", "sv_w", "sv_b",
              "tq_w", "tq_b", "tk_w", "tk_b", "tv_w", "tv_b",
              "s_gamma", "t_gamma"]
    w = {n: np.asarray(inputs[n], np.float32) for n in wnames}

    if "nc" not in _CACHE:
        _CACHE["nc"] = build_program()
    nc = _CACHE["nc"]

    in_maps = [_prep_core_inputs(x_s, w, k) for k in range(8)]
    res = run_bass_kernel_spmd(nc, in_maps, core_ids=list(range(8)))

    out = np.empty((B, L, H, W, C), np.float32)
    for k in range(8):
        o = np.asarray(res.results[k]["out"]).astype(np.float32)  # (2,4096,256)
        b, q = k // 4, k % 4
        for j in range(2):
            out[b, 2 * q + j] = o[j].reshape(H, W, C)
    return out


if __name__ == "__main__":
    import reference as ref
    inputs = {kk: np.asarray(v) for kk, v in ref.setup_inputs().items()}
    expected = np.asarray(ref.reference(**inputs))
    got = kernel(**inputs)
    err = np.abs(got - expected)
    rel = err.max() / np.abs(expected).max()
    print("abs max err:", err.max(), " rel:", float(rel))


# revision 33
# speedup vs baseline: 1.1198x; 1.1198x over previous
"""Trainium2 Bass kernel for nn_LossRecovery (spatial+temporal channel attention).

Sharding: 16 (b,l) slices over 8 cores, 2 slices/core (data-parallel over B*L,
small CxC weights replicated per core), per the sharding hint.

v3 design:
- ONE packed input dram tensor + ONE output tensor per core (per-buffer PJRT
  dispatch overhead ~65us/buffer/call dominates at this kernel size).
- Scores via Gram matrices: scores = Wq.(X.X^T).Wk^T + rank-1 bias
  corrections (bq (x) S_k0 + S_q0 (x) bk + HW bq (x) bk, with S = W.xsum).
  This replaces the q/k convs and all their PSUM->SBUF move+bias traffic
  with a single Gram accumulation per attention. Temporal scores2 use the
  cross-Gram of x1 and x-swapped the same way (softmax logits identical).
- fp16 for all activation tensors from HBM and the output; fp32 PSUM,
  fp32r for the small (256x256) Gram-stage matmuls, fp32 softmax.
- x1 lives only in natural (s,c) bf16 layout; output written natural.
- Engine balance: per-partition biases (v, v2) fused into ACT Identity
  copies; v2p drains split ACT/DVE; residual fuses on DVE.

Temporal-value scramble handling (same math as the previous kernel): v2p row
r=32p+co of slice j holds v2conv[l=(l0+p)%8][band-channel 32j+co]; tk weight
ROWS are permuted host-side (dperm) so attn2 columns match v2p rows directly.
"""
import numpy as np

import concourse.bass as bass
import concourse.bacc as bacc
import concourse.mybir as mybir
import concourse.tile as tile
from concourse.bass_utils import run_bass_kernel_spmd
from concourse.masks import make_identity

B, L, H, W = 2, 8, 64, 64
C, HW = 256, 4096
FP = mybir.dt.float32
BF = mybir.dt.float16
FR = mybir.dt.float32r
NS512 = HW // 512   # 8
NS128 = HW // 128   # 32
AW = 264            # augmented natural-layout width (256 + ones col + pad)
BF16 = np.float16

_CACHE = {}

# ---- packed input layout (f32 words) ----
_LAYOUT = [
    ("wqk32", 256 * 512),       # [sq_w.T | sk_w.T] (256e, 512) f32
    ("wt232", 256 * 512),       # [tq_w.T | tk_w[dperm].T] (256e, 512) f32
    ("wv", 256 * 128),          # sv_w.T (256c, 256d) bf16
    ("wv2", 256 * 32),          # tv_w[band].T (256c, 64) bf16
    ("rows", 1536),             # [bq|bk|bk4096|bq2|bk2|bk2_4096] (1,1536) f32
    ("vb", 256),                # sv_b (256,1) f32
    ("v2b", 64),                # tv_b[band] (64,1) f32
    ("gam", 256),               # gammas broadcast (128,2) f32
    ("idg", 2 * 128 * 64),      # [I/gs ; I/gt] (2,128,128) fp16
    ("xb", 8 * 256 * 2048),     # (8,256,4096) bf16 (c,s), rotated l order
    ("xnat", 2 * 4096 * (AW // 2)),   # (2,4096,264) bf16 nat rows + ones col
    ("xswnat", 2 * 4096 * (AW // 2)), # (2,4096,264) bf16 w-major rows + ones
]
_OFF = {}
_cur = 0
for _n, _sz in _LAYOUT:
    _OFF[_n] = _cur
    _cur += _sz
NWORDS = _cur


def build_program():
    nc = bacc.Bacc("TRN2", target_bir_lowering=False, debug=False, num_devices=8)

    inp = nc.dram_tensor("inp", [NWORDS], FP, kind="ExternalInput")
    out_d = nc.dram_tensor("out", [2, HW, C], BF, kind="ExternalOutput")

    def reg2(name, rows_, wwords):
        o = _OFF[name]
        return inp[o:o + rows_ * wwords].rearrange("(p n) -> p n", p=rows_)

    def reg3(name, rows_, wwords):
        o = _OFF[name]
        return inp[o:o + rows_ * wwords].rearrange(
            "(cc p n) -> p cc n", p=128, n=wwords)

    def xb_chunk(p, cc):
        o = _OFF["xb"] + (p * 256 + cc * 128) * 2048
        return inp[o:o + 128 * 2048].rearrange("(p n) -> p n", p=128)

    def nat_chunks(name, j):
        # (4096, 264w/2) f32 words -> (128, 32, 132) partition-major source AP
        o = _OFF[name] + j * 4096 * (AW // 2)
        return inp[o:o + 4096 * (AW // 2)].rearrange(
            "(s1 p n) -> p s1 n", p=128, n=AW // 2)

    with tile.TileContext(nc) as tc:
        with (
            tc.tile_pool(name="const", bufs=1) as cpool,
            tc.tile_pool(name="big", bufs=1) as big,
            tc.tile_pool(name="swp", bufs=1) as swp,
            tc.tile_pool(name="xbs", bufs=3) as xbs_pool,
            tc.tile_pool(name="gsb", bufs=2) as gsb,
            tc.tile_pool(name="sm", bufs=2) as sm,
            tc.tile_pool(name="rowp", bufs=2) as rowp,
            tc.tile_pool(name="outp", bufs=4) as outp,
            tc.tile_pool(name="psA", bufs=2, space="PSUM") as psA,
            tc.tile_pool(name="psS2", bufs=2, space="PSUM") as psS2,
            tc.tile_pool(name="psO", bufs=2, space="PSUM") as psO,
            tc.tile_pool(name="psG", bufs=2, space="PSUM") as psG,
        ):
            # ---- first-needed loads: xnat0 chunks, v weights ----
            xnat = [big.tile([128, NS128, AW], BF, tag=f"xnat{j}",
                             name=f"xnat{j}") for j in range(2)]
            src0 = nat_chunks("xnat", 0)
            nc.sync.dma_start(xnat[0][:, bass.ts(0, 8), :],
                              src0[:, bass.ts(0, 8), :].bitcast(BF))
            wv = cpool.tile([128, 2, 256], BF, tag="wv")
            nc.sync.dma_start(wv[:], reg3("wv", 256, 128).bitcast(BF))
            wv2 = cpool.tile([128, 2, 64], BF, tag="wv2")
            nc.sync.dma_start(wv2[:], reg3("wv2", 256, 32).bitcast(BF))
            vb = cpool.tile([128, 2, 1], FP, tag="vb")
            nc.sync.dma_start(vb[:], reg3("vb", 256, 1))
            v2b = cpool.tile([64, 1], FP, tag="v2b")
            nc.sync.dma_start(v2b[:], reg2("v2b", 64, 1))
            gam = cpool.tile([128, 2], FP, tag="gam")
            nc.sync.dma_start(gam[:], reg2("gam", 128, 2))
            ident = cpool.tile([128, 128], FP, tag="ident")
            make_identity(nc, ident[:])
            idg = cpool.tile([128, 2, 128], BF, tag="idg")
            nc.sync.dma_start(idg[:], reg3("idg", 256, 64).bitcast(BF))
            for hh in range(1, 4):
                nc.sync.dma_start(xnat[0][:, bass.ts(hh, 8), :],
                                  src0[:, bass.ts(hh, 8), :].bitcast(BF))
            # deferred loads (needed later; emitted after so the DMA queue
            # services the startup-critical tiles first)
            wqk = cpool.tile([128, 2, 512], FR, tag="wqk")
            wt2 = cpool.tile([128, 2, 512], FR, tag="wt2")
            rowsFR = cpool.tile([1, 1536], FR, tag="rowsFR")

            def load_deferred():
                nc.sync.dma_start(wqk[:], reg3("wqk32", 256, 512).bitcast(FR))
                nc.sync.dma_start(rowsFR[:], reg2("rows", 1, 1536).bitcast(FR))
                src1 = nat_chunks("xnat", 1)
                for hh in range(4):
                    nc.sync.dma_start(xnat[1][:, bass.ts(hh, 8), :],
                                      src1[:, bass.ts(hh, 8), :].bitcast(BF))
                nc.sync.dma_start(wt2[:], reg3("wt232", 256, 512).bitcast(FR))

            g_s = gam[:, 0:1]
            g_t = gam[:, 1:2]
            rFR = rowsFR

            x1n = [big.tile([128, NS128, 256], BF, tag=f"x1n{j}",
                            name=f"x1n{j}") for j in range(2)]
            vt = [big.tile([128, 2, HW], BF, tag=f"vt{j}", name=f"vt{j}")
                  for j in range(2)]
            v2p = [big.tile([128, 2, HW], BF, tag=f"v2p{j}", name=f"v2p{j}")
                   for j in range(2)]

            def load_xswnat(j):
                t = swp.tile([128, NS128, AW], BF, tag="xswnat")
                src = nat_chunks("xswnat", j)
                for hh in range(4):
                    nc.sync.dma_start(t[:, bass.ts(hh, 8), :],
                                      src[:, bass.ts(hh, 8), :].bitcast(BF))
                return t

            def emit_gram(lhs_tile, rhs_tile, gpair):
                """gpair[ec] (128,264) psum += lhsT_chunk^T @ rhs over all s.
                rhs col 256 is 1.0 -> col 256 = row sums of lhs. 32 units."""
                for s1 in range(NS128):
                    for ec in range(2):
                        nc.tensor.matmul(
                            gpair[ec][:],
                            lhs_tile[:, s1, bass.ds(128 * ec, 128)],
                            rhs_tile[:, s1, :],
                            start=(s1 == 0), stop=(s1 == NS128 - 1))
                    yield

            def emit_scores_softmax(gpair, wst, roff, attnT, tagn,
                                    temporal=False, xs_sp=None, xs_keep=None):
                """Drain G (+ sum col); S-rows; two (256x256) stages;
                scores += bq(x)R1 + S_q(x)bk; softmax -> attnT.
                wst: (128,2,512) FR = [WqT | WkT] (e-part, echunk, c|d).
                roff: word offset of [bq|bk|bk*HW] inside `rows`.
                Spatial (G symmetric): T1 = G.WkT via lhsT=G, then
                scores = lhsT(WqT-chunk).T1.
                Temporal (G2 = x1^T.Xsw asym): U^T = lhsT(G2).Wq2T, then
                scores2 = lhsT(U^T-chunk).Wk2T; S_k2 row from spatial
                xsum column xs_sp (sum over xsw == sum over x)."""
                Gt = gsb.tile([128, 2, 256], FR, tag="G", name=f"G_{tagn}")
                xs = sm.tile([128, 2, 1], FR, tag="xsum", name=f"xs_{tagn}")
                nc.scalar.copy(Gt[:, 0, :], gpair[0][:, 0:256])
                nc.vector.tensor_copy(Gt[:, 1, :], gpair[1][:, 0:256])
                for ec in range(2):
                    nc.scalar.copy(xs[:, ec, :], gpair[ec][:, 256:257])
                    if xs_keep is not None:
                        nc.scalar.copy(xs_keep[:, ec, :],
                                       gpair[ec][:, 256:257])
                yield
                ps_s = psS2.tile([128, 512], FP, tag="mmS",
                                 name=f"srow_{tagn}")
                if not temporal:
                    # (1,512) = xsum^T . [WqT|WkT] = [S_q row | S_k row]
                    nc.tensor.matmul(ps_s[0:1, :], xs[:, 0, :],
                                     wst[:, 0, :], start=True, stop=False)
                    nc.tensor.matmul(ps_s[0:1, :], xs[:, 1, :],
                                     wst[:, 1, :], start=False, stop=True)
                else:
                    # S_q2 = x1sum^T.Wq2T ; S_k2 = xsum^T.Wk2T
                    nc.tensor.matmul(ps_s[0:1, 0:256], xs[:, 0, :],
                                     wst[:, 0, 0:256], start=True, stop=False)
                    nc.tensor.matmul(ps_s[0:1, 0:256], xs[:, 1, :],
                                     wst[:, 1, 0:256], start=False, stop=True)
                    nc.tensor.matmul(ps_s[0:1, 256:512], xs_sp[:, 0, :],
                                     wst[:, 0, 256:512], start=True, stop=False)
                    nc.tensor.matmul(ps_s[0:1, 256:512], xs_sp[:, 1, :],
                                     wst[:, 1, 256:512], start=False, stop=True)
                sq_row = rowp.tile([1, 256], FR, tag="sqrow",
                                   name=f"sq_{tagn}")
                nc.scalar.copy(sq_row[:], ps_s[0:1, 0:256])
                # R1 = S_k row + HW*bk row
                r1 = rowp.tile([1, 256], FR, tag="r1", name=f"r1_{tagn}")
                nc.vector.tensor_add(r1[:], ps_s[0:1, 256:512],
                                     rowsFR[:, bass.ds(roff + 512, 256)])
                yield
                # stage 1: T1 = lhsT(G).W2 where W2 = WkT (spatial) or
                # WqT (temporal, giving U^T)
                w1off = 256 if not temporal else 0
                t1 = gsb.tile([128, 2, 256], FR, tag="T1", name=f"T1_{tagn}")
                for ec in range(2):
                    pt1 = psS2.tile([128, 512], FP, tag="mmS")
                    nc.tensor.matmul(pt1[:, 0:256],
                                     Gt[:, 0, bass.ds(128 * ec, 128)],
                                     wst[:, 0, bass.ds(w1off, 256)],
                                     start=True, stop=False)
                    nc.tensor.matmul(pt1[:, 0:256],
                                     Gt[:, 1, bass.ds(128 * ec, 128)],
                                     wst[:, 1, bass.ds(w1off, 256)],
                                     start=False, stop=True)
                    if ec == 0:
                        nc.scalar.copy(t1[:, ec, :], pt1[:, 0:256])
                    else:
                        nc.vector.tensor_copy(t1[:, ec, :], pt1[:, 0:256])
                    yield
                # stage 2 + corrections + softmax per c-chunk
                for cc in range(2):
                    psc = psS2.tile([128, 512], FP, tag="mmS")
                    sc = psc[:, 0:256]
                    if not temporal:
                        nc.tensor.matmul(sc,
                                         wst[:, 0, bass.ds(128 * cc, 128)],
                                         t1[:, 0, :],
                                         start=True, stop=False)
                        nc.tensor.matmul(sc,
                                         wst[:, 1, bass.ds(128 * cc, 128)],
                                         t1[:, 1, :],
                                         start=False, stop=False)
                    else:
                        nc.tensor.matmul(sc,
                                         t1[:, 0, bass.ds(128 * cc, 128)],
                                         wst[:, 0, 256:512],
                                         start=True, stop=False)
                        nc.tensor.matmul(sc,
                                         t1[:, 1, bass.ds(128 * cc, 128)],
                                         wst[:, 1, 256:512],
                                         start=False, stop=False)
                    nc.tensor.matmul(sc,
                                     rFR[:, bass.ds(roff + 128 * cc, 128)],
                                     r1[:],
                                     start=False, stop=False)
                    nc.tensor.matmul(sc,
                                     sq_row[:, bass.ds(128 * cc, 128)],
                                     rFR[:, bass.ds(roff + 256, 256)],
                                     start=False, stop=True)
                    mx = sm.tile([128, 1], FP, tag="mx")
                    nc.vector.reduce_max(mx[:], sc, axis=mybir.AxisListType.X)
                    nmx = sm.tile([128, 1], FP, tag="nmx")
                    nc.vector.tensor_scalar_mul(nmx[:], mx[:], -1.0)
                    aexp = sm.tile([128, 256], FP, tag="aexp")
                    ssum = sm.tile([128, 1], FP, tag="ssum")
                    nc.scalar.activation(out=aexp[:], in_=sc,
                                         func=mybir.ActivationFunctionType.Exp,
                                         bias=nmx[:], accum_out=ssum[:])
                    rs = sm.tile([128, 1], FP, tag="rs")
                    nc.vector.reciprocal(rs[:], ssum[:])
                    nc.vector.tensor_scalar_mul(aexp[:], aexp[:], rs[:])
                    for dc in range(2):
                        pt = psO.tile([128, 256], FP, tag="mmO")
                        nc.tensor.transpose(pt[:, 0:128],
                                            aexp[:, bass.ts(dc, 128)],
                                            ident[:])
                        nc.scalar.copy(attnT[:, dc, bass.ts(cc, 128)],
                                       pt[:, 0:128])
                    yield

            def emit_phase0(ps):
                """64-band temporal-v conv for source slices in `ps` -> v2p;
                for p<2 the v conv for slice p is fused in (same stream).
                xb streamed in half-slices. Drains split ACT(j0)/DVE(j1)."""
                for p in ps:
                    for h in range(2):
                        ht = xbs_pool.tile([128, 2, 2048], BF, tag="xbs")
                        for cc in range(2):
                            nc.sync.dma_start(
                                ht[:, cc, :],
                                xb_chunk(p, cc)[:, bass.ds(1024 * h, 1024)
                                                ].bitcast(BF))
                        for sq in range(4):
                            s5 = 4 * h + sq
                            rhs0 = ht[:, 0, bass.ts(sq, 512)]
                            rhs1 = ht[:, 1, bass.ts(sq, 512)]
                            ph = psA.tile([64, 512], FP, tag="mmA")
                            nc.tensor.matmul(ph[:], wv2[:, 0, :], rhs0,
                                             start=True, stop=False)
                            nc.tensor.matmul(ph[:], wv2[:, 1, :], rhs1,
                                             start=False, stop=True)
                            nc.scalar.activation(
                                out=v2p[0][bass.ds(32 * (p % 4), 32), p // 4,
                                           bass.ts(s5, 512)],
                                in_=ph[0:32, :],
                                func=mybir.ActivationFunctionType.Identity,
                                bias=v2b[0:32, 0:1])
                            nc.vector.tensor_scalar_add(
                                v2p[1][bass.ds(32 * (p % 4), 32), p // 4,
                                       bass.ts(s5, 512)],
                                ph[32:64, :], v2b[32:64, 0:1])
                            if p < 2:
                                for dc in range(2):
                                    pv = psA.tile([128, 512], FP, tag="mmA")
                                    nc.tensor.matmul(
                                        pv[:], wv[:, 0, bass.ts(dc, 128)],
                                        rhs0, start=True, stop=False)
                                    nc.tensor.matmul(
                                        pv[:], wv[:, 1, bass.ts(dc, 128)],
                                        rhs1, start=False, stop=True)
                                    nc.vector.tensor_scalar_add(
                                        vt[p][:, dc, bass.ts(s5, 512)],
                                        pv[:], vb[:, dc, 0:1])
                            yield

            def emit_out1(j, attnT):
                """spatial out (s,c) + residual -> x1n[j] (fp16). 32 units.
                Alternates drain engine per chunk: even -> DVE fused stt;
                odd -> residual I/gs matmul + ACT scale-by-gs copy."""
                for s1 in range(NS128):
                    po = psO.tile([128, 256], FP, tag="mmO")
                    act_side = s1 % 2 == 1
                    if act_side:
                        nc.tensor.matmul(po[:], idg[:, 0, :],
                                         xnat[j][:, s1, 0:256],
                                         start=True, stop=False)
                    nc.tensor.matmul(po[:],
                                     vt[j][:, 0, bass.ts(s1, 128)],
                                     attnT[:, 0, :], start=not act_side,
                                     stop=False)
                    nc.tensor.matmul(po[:],
                                     vt[j][:, 1, bass.ts(s1, 128)],
                                     attnT[:, 1, :], start=False, stop=True)
                    if act_side:
                        nc.scalar.activation(
                            out=x1n[j][:, s1, :], in_=po[:],
                            func=mybir.ActivationFunctionType.Identity,
                            scale=g_s)
                    else:
                        nc.vector.scalar_tensor_tensor(
                            out=x1n[j][:, s1, :], in0=po[:], scalar=g_s,
                            in1=xnat[j][:, s1, 0:256],
                            op0=mybir.AluOpType.mult,
                            op1=mybir.AluOpType.add)
                    yield

            def emit_out2(j, attn2T):
                """temporal out (s,c) + residual -> x2 -> DMA out. 32 units.
                Same alternating drain-engine scheme as emit_out1."""
                for s1 in range(NS128):
                    po = psO.tile([128, 256], FP, tag="mmO")
                    act_side = s1 % 2 == 1
                    if act_side:
                        nc.tensor.matmul(po[:], idg[:, 1, :],
                                         x1n[j][:, s1, :],
                                         start=True, stop=False)
                    nc.tensor.matmul(po[:],
                                     v2p[j][:, 0, bass.ts(s1, 128)],
                                     attn2T[:, 0, :], start=not act_side,
                                     stop=False)
                    nc.tensor.matmul(po[:],
                                     v2p[j][:, 1, bass.ts(s1, 128)],
                                     attn2T[:, 1, :], start=False, stop=True)
                    x2 = outp.tile([128, 256], BF, tag="x2")
                    if act_side:
                        nc.scalar.activation(
                            out=x2[:], in_=po[:],
                            func=mybir.ActivationFunctionType.Identity,
                            scale=g_t)
                    else:
                        nc.vector.scalar_tensor_tensor(
                            out=x2[:], in0=po[:], scalar=g_t,
                            in1=x1n[j][:, s1, :],
                            op0=mybir.AluOpType.mult,
                            op1=mybir.AluOpType.add)
                    nc.sync.dma_start(
                        out_d[j, bass.ds(128 * s1, 128), :], x2[:])
                    yield

            def weave(a, b, ratio=1):
                ita, itb = iter(a), iter(b)
                alive_a = alive_b = True
                while alive_a or alive_b:
                    if alive_a:
                        try:
                            next(ita)
                        except StopIteration:
                            alive_a = False
                    if alive_b:
                        for _ in range(ratio):
                            try:
                                next(itb)
                            except StopIteration:
                                alive_b = False
                                break

            def chain(*gens):
                for g in gens:
                    yield from g

            def drain(g):
                for _ in g:
                    pass

            # ================= emission order =================
            gA0 = [psG.tile([128, AW], FP, tag="gram", name=f"gA0_{e}")
                   for e in range(2)]
            weave(emit_phase0(range(2)), emit_gram(xnat[0], xnat[0], gA0), 2)
            load_deferred()
            attnT0 = sm.tile([128, 2, 256], BF, tag="attnT", name="attnT0")
            xs_s0 = sm.tile([128, 2, 1], FR, tag="xsC", name="xsc_s0")
            ph0 = emit_phase0(range(2, L))
            weave(emit_scores_softmax(gA0, wqk, 0, attnT0, "s0",
                                      xs_keep=xs_s0), ph0, 2)
            gA1 = [psG.tile([128, AW], FP, tag="gram", name=f"gA1_{e}")
                   for e in range(2)]
            weave(emit_gram(xnat[1], xnat[1], gA1),
                  chain(emit_out1(0, attnT0), ph0), 1)
            attnT1 = sm.tile([128, 2, 256], BF, tag="attnT", name="attnT1")
            xs_s1 = sm.tile([128, 2, 1], FR, tag="xsC", name="xsc_s1")
            weave(emit_scores_softmax(gA1, wqk, 0, attnT1, "s1",
                                      xs_keep=xs_s1), ph0, 2)
            xsw0 = load_xswnat(0)
            gB0 = [psG.tile([128, AW], FP, tag="gram", name=f"gB0_{e}")
                   for e in range(2)]
            o11 = emit_out1(1, attnT1)
            weave(emit_gram(x1n[0], xsw0, gB0), o11, 1)
            attn2T0 = sm.tile([128, 2, 256], BF, tag="attnT", name="attn2T0")
            weave(emit_scores_softmax(gB0, wt2, 768, attn2T0, "t0",
                                      temporal=True, xs_sp=xs_s0),
                  chain(o11, ph0), 2)
            xsw1 = load_xswnat(1)
            o20 = emit_out2(0, attn2T0)
            gB1 = [psG.tile([128, AW], FP, tag="gram", name=f"gB1_{e}")
                   for e in range(2)]
            weave(emit_gram(x1n[1], xsw1, gB1), chain(ph0, o20), 1)
            attn2T1 = sm.tile([128, 2, 256], BF, tag="attnT", name="attn2T1")
            weave(emit_scores_softmax(gB1, wt2, 768, attn2T1, "t1",
                                      temporal=True, xs_sp=xs_s1),
                  chain(ph0, o20), 2)
            drain(ph0)
            weave(emit_out2(1, attn2T1), o20, 1)

    nc.compile()
    return nc


def _tobf_words(a):
    """float32 array -> fp16 packed into f32 words (last dim halved)."""
    b = np.ascontiguousarray(np.asarray(a, np.float32)).astype(BF16)
    return b.view(np.float32)


def _aug(nat):
    """(4096,256) -> (4096,264): ones col at 256, zero pad."""
    out = np.zeros((HW, AW), np.float32)
    out[:, :C] = nat
    out[:, C] = 1.0
    return out


def _prep_core_inputs(x_s, w, k):
    """Host-side packing for core k. x_s: (2,8,64,64,256) fp32."""
    b, q = k // 4, k % 4
    l0 = 2 * q
    band = 32 * l0
    rr = np.arange(C)
    dperm = 8 * (rr % 32) + ((l0 + rr // 32) % 8)
    xb_full = x_s[b]  # (8,64,64,256)
    f32 = np.float32
    rows = np.concatenate([
        w["sq_b"], w["sk_b"], float(HW) * w["sk_b"],
        w["tq_b"], w["tk_b"][dperm],
        float(HW) * w["tk_b"][dperm]]).reshape(1, -1)
    pieces = {
        "wqk32": np.ascontiguousarray(
            np.concatenate([w["sq_w"].T, w["sk_w"].T], axis=1), f32),
        "wt232": np.ascontiguousarray(
            np.concatenate([w["tq_w"].T, w["tk_w"][dperm].T], axis=1), f32),
        "wv": _tobf_words(w["sv_w"].T),
        "wv2": _tobf_words(w["tv_w"][band:band + 64].T),
        "rows": np.ascontiguousarray(rows, f32),
        "vb": np.ascontiguousarray(w["sv_b"].reshape(C, 1), f32),
        "v2b": np.ascontiguousarray(
            w["tv_b"][band:band + 64].reshape(64, 1), f32),
        "gam": np.ascontiguousarray(np.broadcast_to(
            np.stack([w["s_gamma"][0], w["t_gamma"][0]]), (128, 2)), f32),
        "idg": _tobf_words(np.stack(
            [np.eye(128, dtype=f32) / w["s_gamma"][0],
             np.eye(128, dtype=f32) / w["t_gamma"][0]])),
        # xb rotated: row p holds slice l=(l0+p)%8 in (c,s) layout
        "xb": _tobf_words(np.stack(
            [xb_full[(l0 + p) % 8].transpose(2, 0, 1).reshape(C, HW)
             for p in range(L)])),
        "xnat": _tobf_words(np.stack(
            [_aug(xb_full[l].reshape(HW, C)) for l in (l0, l0 + 1)])),
        "xswnat": _tobf_words(np.stack(
            [_aug(xb_full[l].transpose(1, 0, 2).reshape(HW, C))
             for l in (l0, l0 + 1)])),
    }
    buf = np.empty(NWORDS, f32)
    for name, sz in _LAYOUT:
        o = _OFF[name]
        buf[o:o + sz] = pieces[name].reshape(-1)
    return {"inp": buf}


def kernel(**inputs):
    x = np.asarray(inputs["x"], np.float32)
    x_s = np.ascontiguousarray(x[..., :C])
    wnames = ["sq_w", "sq_b", "sk_w", "sk_b", "sv_w", "sv_b",
              "tq_w", "tq_b", "tk_w", "tk_b", "tv_w", "tv_b",
              "s_gamma", "t_gamma"]
    w = {n: np.asarray(inputs[n], np.float32) for n in wnames}

    if "nc" not in _CACHE:
        _CACHE["nc"] = build_program()
    nc = _CACHE["nc"]

    in_maps = [_prep_core_inputs(x_s, w, k) for k in range(8)]
    res = run_bass_kernel_spmd(nc, in_maps, core_ids=list(range(8)))

    out = np.empty((B, L, H, W, C), np.float32)
    for k in range(8):
        o = np.asarray(res.results[k]["out"]).astype(np.float32)  # (2,4096,256)
        b, q = k // 4, k % 4
        for j in range(2):
            out[b, 2 * q + j] = o[j].reshape(H, W, C)
    return out


if __name__ == "__main__":
    import reference as ref
    inputs = {kk: np.asarray(v) for kk, v in ref.setup_inputs().items()}
    expected = np.asarray(ref.reference(**inputs))
    got = kernel(**inputs)
    err = np.abs(got - expected)
    rel = err.max() / np.abs(expected).max()
    print("abs max err:", err.max(), " rel:", float(rel))
